# revision 17
# baseline (speedup 1.0000x reference)
"""Trainium2 Bass kernel: fused multi-head causal self-attention block.

Computes, for x:(B,S,H), W_qkv:(3H,H), b_qkv:(3H,), W_out:(H,H), b_out:(H,):
    qkv = x @ W_qkv.T + b_qkv ; split into q,k,v heads (NH heads, D=H/NH)
    out = softmax(causal(q k^T / sqrt(D))) v   ; merge heads
    return out @ W_out.T + b_out

Sharding over 8 NeuronCores: DP(2 batches) x TP(4 head-groups).
Core c handles batch b=c//4, head group g=c%4 (heads 4g..4g+3).

Schedule per core:
  - x chunks + all weights DMA'd into SBUF once (fp16).
  - A2: V projection for all 4 local heads.
  - per head h: Q/K projection -> causal attention -> AllGather of A^T
    (fp16) within the 4-core batch group; collectives overlap the
    following heads' compute.
  - 4 out-projection passes at the end (each only needs its own
    AllGather, so the last collective hides under the first 3 passes).
Host does a pure concatenation of the per-core [S, 512] output slices.

All matmuls run 16-bit (fp16 operands; bf16 for exp-weights/V);
accumulation is fp32 in PSUM. Diagonal attention tiles are
width-restricted (causal) to skip fully-masked columns.
"""

import math

import numpy as np
import ml_dtypes

import concourse.bass as bass
import concourse.mybir as mybir
import concourse.tile as tile
from concourse import bacc
from concourse.bass_utils import run_bass_kernel_spmd

FP = mybir.dt.float32
FR = mybir.dt.float32r
F16 = mybir.dt.float16
BF = mybir.dt.bfloat16

# Full-size problem constants.
B, S, H, NH = 2, 2048, 2048, 16
D = 128
NCORES = 8
GROUPS = 4                  # head-groups per batch (TP degree)
REPLICA_GROUPS = [[0, 1, 2, 3], [4, 5, 6, 7]]

TRACE = False               # set by test harness to capture NTFF profile
LAST_EXEC_NS = None
LAST_RESULTS = None

NL = NH // GROUPS           # local heads per core (4)
DG = NL * D                 # per-core slice of the head dim (512)
HC = H // 128               # contraction chunks (16)
SQ = S // 512               # 512-wide strips (4)
ST_N = S // 128             # 128-row s tiles (16)


def build_nc():
    """Build the SPMD Bass program (identical on all 8 cores)."""
    scale = 1.0 / math.sqrt(D)

    nc = bacc.Bacc(
        "TRN2",
        target_bir_lowering=False,
        debug=False,
        enable_asserts=False,
        num_devices=NCORES,
    )

    # ---- I/O -----------------------------------------------------------
    xT_d = nc.dram_tensor("xT", [H, S], F16, kind="ExternalInput")
    wq_d = nc.dram_tensor("wq", [H, DG], F16, kind="ExternalInput")
    wk_d = nc.dram_tensor("wk", [H, DG], F16, kind="ExternalInput")
    wv_d = nc.dram_tensor("wv", [H, DG], F16, kind="ExternalInput")
    wo_d = nc.dram_tensor("wo", [H, DG], F16, kind="ExternalInput")
    bq_d = nc.dram_tensor("bq", [128, NL], FP, kind="ExternalInput")
    bk_d = nc.dram_tensor("bk", [128, NL], FP, kind="ExternalInput")
    bv_d = nc.dram_tensor("bv", [128, DG], FP, kind="ExternalInput")
    bo_d = nc.dram_tensor("bo", [128, DG], FP, kind="ExternalInput")
    mask_d = nc.dram_tensor("mask", [128, 128], BF, kind="ExternalInput")
    out_d = nc.dram_tensor("out", [S, DG], FP, kind="ExternalOutput")

    with tile.TileContext(nc) as tc:
        with tc.tile_pool(name="const", bufs=1) as constp:
            # constants ride the scalar DMA queue so the sync queue's first
            # transfers are the wv/x chunks the first matmul waits on
            mask_sb = constp.tile([128, 128], BF)
            nc.scalar.dma_start(mask_sb[:], mask_d[:])
            bq_sb = constp.tile([128, NL], FP)
            nc.scalar.dma_start(bq_sb[:], bq_d[:])
            bk_sb = constp.tile([128, NL], FP)
            nc.scalar.dma_start(bk_sb[:], bk_d[:])
            bv_sb = constp.tile([128, DG], FP)
            nc.scalar.dma_start(bv_sb[:], bv_d[:])
            bo_sb = constp.tile([128, DG], FP)
            nc.scalar.dma_start(bo_sb[:], bo_d[:])
            ones_sb = constp.tile([128, 128], BF)
            nc.vector.memset(ones_sb[:], 1.0)

            _emit_body(nc, tc, scale,
                       xT_d, wq_d, wk_d, wv_d, wo_d, out_d,
                       bq_sb, bk_sb, bv_sb, bo_sb, mask_sb, ones_sb)

    nc.compile()
    return nc


def _emit_body(nc, tc, scale,
               xT_d, wq_d, wk_d, wv_d, wo_d, out_d,
               bq_sb, bk_sb, bv_sb, bo_sb, mask_sb, ones_sb):
    with tc.tile_pool(name="vv", bufs=1) as vvp, \
         tc.tile_pool(name="dramp", bufs=1, space="DRAM") as dramp:
        vv = [vvp.tile([128, DG], BF, tag=f"v{t}", name=f"v{t}")
              for t in range(ST_N)]
        agouts = _emit_proj_attn(nc, tc, scale, xT_d, wq_d, wk_d, wv_d,
                                 bq_sb, bk_sb, bv_sb, mask_sb, ones_sb, vv,
                                 dramp)
        _emit_outproj(nc, tc, wo_d, out_d, bo_sb, agouts)


def _emit_proj_attn(nc, tc, scale, xT_d, wq_d, wk_d, wv_d,
                    bq_sb, bk_sb, bv_sb, mask_sb, ones_sb, vv, dramp):
    # x chunks + projection weights resident only for this region; the
    # pool closes before the out-projection so its SBUF is reusable there.
    with tc.tile_pool(name="xw", bufs=1) as xwp:
        xsb = xwp.tile([128, HC, S], F16, name="xsb")
        wvsb = xwp.tile([128, HC, DG], F16, name="wvsb")
        wqsb = xwp.tile([128, HC, DG], F16, name="wqsb")
        wksb = xwp.tile([128, HC, DG], F16, name="wksb")
        # DMA order: wv + x strip0 interleaved (A2 starts ~1.5us in),
        # then x strips 1-3, then q/k weights (needed ~55us in).
        # First pair at single-chunk granularity so A2's first matmul
        # only waits on 256KB.
        for c in range(4):
            rows = slice(128 * c, 128 * c + 128)
            nc.sync.dma_start(
                wvsb[:, c:c + 1, :],
                wv_d[rows, :].rearrange("(c p) d -> p c d", p=128))
            nc.sync.dma_start(
                xsb[:, c:c + 1, 0:512],
                xT_d[rows, 0:512].rearrange("(c p) t -> p c t", p=128))
        for hb in range(1, 4):
            rows = slice(512 * hb, 512 * hb + 512)
            nc.sync.dma_start(
                wvsb[:, 4 * hb:4 * hb + 4, :],
                wv_d[rows, :].rearrange("(c p) d -> p c d", p=128))
            nc.sync.dma_start(
                xsb[:, 4 * hb:4 * hb + 4, 0:512],
                xT_d[rows, 0:512].rearrange("(c p) t -> p c t", p=128))
        for strip in range(1, SQ):
            cs = slice(512 * strip, 512 * strip + 512)
            for hb in range(4):
                rows = slice(512 * hb, 512 * hb + 512)
                nc.sync.dma_start(
                    xsb[:, 4 * hb:4 * hb + 4, cs],
                    xT_d[rows, cs].rearrange("(c p) t -> p c t", p=128))
        # q/k weights per head (head 0's slice first, so qk0 never waits)
        for h in range(NL):
            hs = slice(128 * h, 128 * h + 128)
            nc.sync.dma_start(
                wqsb[:, :, hs],
                wq_d[:, hs].rearrange("(c p) d -> p c d", p=128))
            nc.sync.dma_start(
                wksb[:, :, hs],
                wk_d[:, hs].rearrange("(c p) d -> p c d", p=128))

        # ---- A2: V projection (natural [s, d] layout, all heads) -------
        with tc.tile_pool(name="psV", bufs=2, space="PSUM") as psV:
            for strip in range(SQ):
                psv = [psV.tile([128, DG], FP, tag=f"psv{i}", name=f"psv{i}")
                       for i in range(4)]
                for c in range(HC):
                    for i in range(4):
                        tok = slice(512 * strip + 128 * i,
                                    512 * strip + 128 * i + 128)
                        nc.tensor.matmul(
                            psv[i][:],
                            xsb[:, c, tok],
                            wvsb[:, c, :],
                            start=(c == 0), stop=(c == HC - 1),
                        )
                for i in range(4):
                    nc.vector.tensor_add(vv[4 * strip + i][:], psv[i][:], bv_sb[:])

        # ---- per-head: Q/K projection + attention + AllGather ----------
        agouts = []
        with tc.tile_pool(name="qkt", bufs=1) as qktp, \
             tc.tile_pool(name="etp", bufs=5) as etp, \
             tc.tile_pool(name="atp", bufs=3) as atp, \
             tc.tile_pool(name="rbp", bufs=2) as rbp, \
             tc.tile_pool(name="psQK", bufs=1, space="PSUM") as psQK, \
             tc.tile_pool(name="psS", bufs=2, space="PSUM") as psS, \
             tc.tile_pool(name="psAV", bufs=2, space="PSUM") as psAV, \
             tc.tile_pool(name="psDN", bufs=2, space="PSUM") as psDN:

            for h in range(NL):
                hs = slice(128 * h, 128 * h + 128)
                qT = qktp.tile([128, S], F16, tag="qT", name="qT")
                kT = qktp.tile([128, S], F16, tag="kT", name="kT")

                # Q/K projection for this head (q chunks then k chunks per
                # strip, so each PSUM bank's drain hides under the other's
                # matmuls)
                for strip in range(SQ):
                    cs = slice(512 * strip, 512 * strip + 512)
                    psq = psQK.tile([128, 512], FP, tag="psq", name="psq")
                    for c in range(HC):
                        nc.tensor.matmul(
                            psq[:], wqsb[:, c, hs], xsb[:, c, cs],
                            start=(c == 0), stop=(c == HC - 1),
                        )
                    nc.scalar.activation(
                        qT[:, cs], psq[:],
                        mybir.ActivationFunctionType.Identity,
                        bias=bq_sb[:, h:h + 1],
                    )
                    psk = psQK.tile([128, 512], FP, tag="psk", name="psk")
                    for c in range(HC):
                        nc.tensor.matmul(
                            psk[:], wksb[:, c, hs], xsb[:, c, cs],
                            start=(c == 0), stop=(c == HC - 1),
                        )
                    nc.scalar.activation(
                        kT[:, cs], psk[:],
                        mybir.ActivationFunctionType.Identity,
                        bias=bk_sb[:, h:h + 1],
                    )

                # causal attention for this head
                for qs in range(SQ):
                    cs = slice(512 * qs, 512 * qs + 512)
                    nk = 4 * qs + 4
                    ps_av = psAV.tile([128, 512], FP, tag="ps_av", name="ps_av")
                    ps_dn = psDN.tile([128, 512], FP, tag="ps_dn", name="ps_dn")
                    for kt in range(nk):
                        j = kt - 4 * qs        # >=0: diagonal (partial) tile
                        off = 128 * j if j >= 0 else 0
                        qsl = slice(512 * qs + off, 512 * qs + 512)
                        w = 512 - off
                        ps_s = psS.tile([128, 512], FP, tag="ps_s", name="ps_s")
                        nc.tensor.matmul(
                            ps_s[:, off:512],
                            kT[:, 128 * kt:128 * kt + 128],
                            qT[:, qsl],
                            start=True, stop=True,
                        )
                        et = etp.tile([128, 512], BF, tag="et", name="et")
                        nc.scalar.activation(
                            et[:, off:512], ps_s[:, off:512],
                            mybir.ActivationFunctionType.Exp,
                            scale=scale,
                        )
                        if j >= 0:  # causal triangle on the leading 128 cols
                            nc.vector.tensor_mul(
                                et[:, off:off + 128], et[:, off:off + 128],
                                mask_sb[:])
                        nc.tensor.matmul(
                            ps_dn[:, off:512], ones_sb[:], et[:, off:512],
                            start=(kt == 0), stop=(kt == nk - 1),
                        )
                        nc.tensor.matmul(
                            ps_av[:, off:512], vv[kt][:, hs], et[:, off:512],
                            start=(kt == 0), stop=(kt == nk - 1),
                        )
                    # normalize: an = ps_av / denom (denom broadcast across
                    # partitions by the ones[128,128] lhsT of the dn matmul)
                    rb = rbp.tile([128, 512], FP, tag="rb", name="rb")
                    nc.vector.reciprocal_approx_fast(rb[:], ps_dn[:])
                    an = atp.tile([128, 512], F16, tag="an", name="an")
                    nc.vector.tensor_mul(an[:], ps_av[:], rb[:])
                    # per-strip AllGather: pieces ship while later strips /
                    # heads still compute, so the last (small) piece lands
                    # shortly after the final an instead of a whole-head
                    # collective gating the last out-projection.
                    agin = dramp.tile([128, 512], F16,
                                      tag=f"agin{h}_{qs}", name=f"agin{h}_{qs}")
                    nc.sync.dma_start(agin[:], an[:])
                    agout = dramp.tile([512, 512], F16,
                                       tag=f"agout{h}_{qs}", name=f"agout{h}_{qs}")
                    nc.gpsimd.collective_compute(
                        "AllGather",
                        mybir.AluOpType.bypass,
                        replica_groups=REPLICA_GROUPS,
                        ins=[agin.opt()],
                        outs=[agout.opt()],
                    )
                    agouts.append(agout)  # index h*SQ + qs
    return agouts


def _emit_outproj(nc, tc, wo_d, out_d, bo_sb, agouts):
    # 4 passes, each over its own head's AllGather pieces. Piece loads and
    # wo loads ride the scalar engine's DMA queue so they don't serialize
    # behind the attention an-DMAs on the sync queue; each [128,4,512]
    # piece staged once serves 4 token tiles.
    with tc.tile_pool(name="wop", bufs=1) as wop, \
         tc.tile_pool(name="oaccp", bufs=1) as oaccp, \
         tc.tile_pool(name="piecep", bufs=4) as piecep, \
         tc.tile_pool(name="outp", bufs=4) as outp, \
         tc.tile_pool(name="psO", bufs=4, space="PSUM") as psO:
        oacc = [oaccp.tile([128, DG], FP, tag=f"oacc{i}", name=f"oacc{i}")
                for i in range(ST_N)]
        wo4s = []
        for l in range(NL):
            wo4 = wop.tile([128, 4, DG], F16, tag=f"wo{l}", name=f"wo{l}")
            nc.scalar.dma_start(
                wo4[:],
                wo_d[512 * l:512 * l + 512, :].rearrange("(c p) d -> p c d", p=128))
            wo4s.append(wo4)
        for l in range(NL):
            last = (l == NL - 1)
            wo4 = wo4s[l]
            for ps in range(SQ):
                pc = piecep.tile([128, 4, 512], F16, tag="pc", name="pc")
                nc.scalar.dma_start(
                    pc[:],
                    agouts[SQ * l + ps][:].rearrange("(r p) t -> p r t", p=128))
                for si in range(4):
                    sti = 4 * ps + si
                    rs = slice(128 * sti, 128 * sti + 128)
                    lcl = slice(128 * si, 128 * si + 128)
                    ps_o = psO.tile([128, DG], FP, tag="ps_o", name="ps_o")
                    for r in range(4):
                        nc.tensor.matmul(
                            ps_o[:], pc[:, r, lcl], wo4[:, r, :],
                            start=(r == 0), stop=(r == 3),
                        )
                    if l == 0:
                        nc.vector.tensor_add(oacc[sti][:], ps_o[:], bo_sb[:])
                    elif not last:
                        nc.vector.tensor_add(oacc[sti][:], ps_o[:], oacc[sti][:])
                    else:
                        ob = outp.tile([128, DG], FP, tag="ob", name="ob")
                        nc.vector.tensor_add(ob[:], ps_o[:], oacc[sti][:])
                        nc.sync.dma_start(out_d[rs, :], ob[:])


def make_inputs(x, W_qkv, b_qkv, W_out, b_out):
    """Host-side sharding: per-core input dicts."""
    x = np.asarray(x, dtype=np.float32)
    W_qkv = np.asarray(W_qkv, dtype=np.float32)
    b_qkv = np.asarray(b_qkv, dtype=np.float32)
    W_out = np.asarray(W_out, dtype=np.float32)
    b_out = np.asarray(b_out, dtype=np.float32)

    # lower-triangle mask block: mask[i, t] = 1 iff t >= i
    tt = np.arange(128)[None, :]
    ii = np.arange(128)[:, None]
    mask = (tt >= ii).astype(ml_dtypes.bfloat16)

    WoT = W_out.T  # [h (d-in), h (n-out)]
    in_maps = []
    for c in range(NCORES):
        b, g = divmod(c, GROUPS)
        xT = np.ascontiguousarray(x[b].T.astype(np.float16))    # [h, s]
        wq = np.ascontiguousarray(
            W_qkv[DG * g:DG * (g + 1), :].T.astype(np.float16))
        wk = np.ascontiguousarray(
            W_qkv[H + DG * g:H + DG * (g + 1), :].T.astype(np.float16))
        wv = np.ascontiguousarray(
            W_qkv[2 * H + DG * g:2 * H + DG * (g + 1), :].T.astype(np.float16))
        bq = np.ascontiguousarray(
            b_qkv[DG * g:DG * (g + 1)].reshape(NL, 128).T)      # [128, nl]
        bk = np.ascontiguousarray(
            b_qkv[H + DG * g:H + DG * (g + 1)].reshape(NL, 128).T)
        bv = np.tile(b_qkv[2 * H + DG * g:2 * H + DG * (g + 1)][None, :], (128, 1))
        bo = np.tile(b_out[DG * g:DG * (g + 1)][None, :], (128, 1))
        # W_out^T rows permuted to the AllGather d-order:
        # ci = l*4 + r  ->  global head 4r + l (within this batch group)
        blocks = []
        for l in range(NL):
            for r in range(GROUPS):
                hh = NL * r + l  # head held as local-head l by group-rank r
                blocks.append(WoT[D * hh:D * (hh + 1), DG * g:DG * (g + 1)])
        wo = np.ascontiguousarray(
            np.concatenate(blocks, axis=0).astype(np.float16))  # [h, dg] fp16
        in_maps.append({
            "xT": xT, "wq": wq, "wk": wk, "wv": wv, "wo": wo,
            "bq": bq, "bk": bk,
            "bv": np.ascontiguousarray(bv), "bo": np.ascontiguousarray(bo),
            "mask": mask,
        })
    return in_maps


_NC_CACHE = {}


def _get_nc():
    if "nc" not in _NC_CACHE:
        _NC_CACHE["nc"] = build_nc()
    return _NC_CACHE["nc"]


def kernel(x, W_qkv, b_qkv, W_out, b_out):
    global LAST_EXEC_NS, LAST_RESULTS
    nc = _get_nc()
    in_maps = make_inputs(x, W_qkv, b_qkv, W_out, b_out)
    res = run_bass_kernel_spmd(
        nc, in_maps, core_ids=list(range(NCORES)), trace=TRACE)
    LAST_EXEC_NS = res.exec_time_ns
    LAST_RESULTS = res
    out = np.empty((B, S, H), dtype=np.float32)
    for c in range(NCORES):
        b, g = divmod(c, GROUPS)
        out[b, :, DG * g:DG * (g + 1)] = res.results[c]["out"]
    return out


# revision 18
# speedup vs baseline: 1.0254x; 1.0254x over previous
"""Trainium2 Bass kernel: fused multi-head causal self-attention block.

Computes, for x:(B,S,H), W_qkv:(3H,H), b_qkv:(3H,), W_out:(H,H), b_out:(H,):
    qkv = x @ W_qkv.T + b_qkv ; split into q,k,v heads (NH heads, D=H/NH)
    out = softmax(causal(q k^T / sqrt(D))) v   ; merge heads
    return out @ W_out.T + b_out

Sharding over 8 NeuronCores: DP(2 batches) x TP(4 head-groups).
Core c handles batch b=c//4, head group g=c%4 (heads 4g..4g+3).

Schedule per core:
  - x chunks + all weights DMA'd into SBUF once (fp16).
  - A2: V projection for all 4 local heads.
  - per head h: Q/K projection -> causal attention -> AllGather of A^T
    (fp16) within the 4-core batch group; collectives overlap the
    following heads' compute.
  - 4 out-projection passes at the end (each only needs its own
    AllGather, so the last collective hides under the first 3 passes).
Host does a pure concatenation of the per-core [S, 512] output slices.

All matmuls run 16-bit (fp16 operands; bf16 for exp-weights/V);
accumulation is fp32 in PSUM. Diagonal attention tiles are
width-restricted (causal) to skip fully-masked columns.
"""

import math

import numpy as np
import ml_dtypes

import concourse.bass as bass
import concourse.mybir as mybir
import concourse.tile as tile
from concourse import bacc
from concourse.bass_utils import run_bass_kernel_spmd

FP = mybir.dt.float32
FR = mybir.dt.float32r
F16 = mybir.dt.float16
BF = mybir.dt.bfloat16

# Full-size problem constants.
B, S, H, NH = 2, 2048, 2048, 16
D = 128
NCORES = 8
GROUPS = 4                  # head-groups per batch (TP degree)
REPLICA_GROUPS = [[0, 1, 2, 3], [4, 5, 6, 7]]

TRACE = False               # set by test harness to capture NTFF profile
LAST_EXEC_NS = None
LAST_RESULTS = None

NL = NH // GROUPS           # local heads per core (4)
DG = NL * D                 # per-core slice of the head dim (512)
HC = H // 128               # contraction chunks (16)
SQ = S // 512               # 512-wide strips (4)
ST_N = S // 128             # 128-row s tiles (16)


def build_nc():
    """Build the SPMD Bass program (identical on all 8 cores)."""
    scale = 1.0 / math.sqrt(D)

    nc = bacc.Bacc(
        "TRN2",
        target_bir_lowering=False,
        debug=False,
        enable_asserts=False,
        num_devices=NCORES,
    )

    # ---- I/O -----------------------------------------------------------
    xT_d = nc.dram_tensor("xT", [H, S], F16, kind="ExternalInput")
    wq_d = nc.dram_tensor("wq", [H, DG], F16, kind="ExternalInput")
    wk_d = nc.dram_tensor("wk", [H, DG], F16, kind="ExternalInput")
    wv_d = nc.dram_tensor("wv", [H, DG], F16, kind="ExternalInput")
    wo_d = nc.dram_tensor("wo", [H, DG], F16, kind="ExternalInput")
    bq_d = nc.dram_tensor("bq", [128, NL], FP, kind="ExternalInput")
    bk_d = nc.dram_tensor("bk", [128, NL], FP, kind="ExternalInput")
    bv_d = nc.dram_tensor("bv", [128, DG], FP, kind="ExternalInput")
    bo_d = nc.dram_tensor("bo", [128, DG], FP, kind="ExternalInput")
    mask_d = nc.dram_tensor("mask", [128, 128], BF, kind="ExternalInput")
    out_d = nc.dram_tensor("out", [S, DG], FP, kind="ExternalOutput")

    with tile.TileContext(nc) as tc:
        with tc.tile_pool(name="const", bufs=1) as constp:
            # constants ride the scalar DMA queue so the sync queue's first
            # transfers are the wv/x chunks the first matmul waits on
            mask_sb = constp.tile([128, 128], BF)
            nc.scalar.dma_start(mask_sb[:], mask_d[:])
            bq_sb = constp.tile([128, NL], FP)
            nc.scalar.dma_start(bq_sb[:], bq_d[:])
            bk_sb = constp.tile([128, NL], FP)
            nc.scalar.dma_start(bk_sb[:], bk_d[:])
            bv_sb = constp.tile([128, DG], FP)
            nc.scalar.dma_start(bv_sb[:], bv_d[:])
            bo_sb = constp.tile([128, DG], FP)
            nc.scalar.dma_start(bo_sb[:], bo_d[:])
            ones_sb = constp.tile([128, 128], BF)
            nc.vector.memset(ones_sb[:], 1.0)

            _emit_body(nc, tc, scale,
                       xT_d, wq_d, wk_d, wv_d, wo_d, out_d,
                       bq_sb, bk_sb, bv_sb, bo_sb, mask_sb, ones_sb)

    nc.compile()
    return nc


def _emit_body(nc, tc, scale,
               xT_d, wq_d, wk_d, wv_d, wo_d, out_d,
               bq_sb, bk_sb, bv_sb, bo_sb, mask_sb, ones_sb):
    with tc.tile_pool(name="vv", bufs=1) as vvp, \
         tc.tile_pool(name="dramp", bufs=1, space="DRAM") as dramp:
        vv = [vvp.tile([128, DG], BF, tag=f"v{t}", name=f"v{t}")
              for t in range(ST_N)]
        agouts = _emit_proj_attn(nc, tc, scale, xT_d, wq_d, wk_d, wv_d,
                                 bq_sb, bk_sb, bv_sb, mask_sb, ones_sb, vv,
                                 dramp)
        _emit_outproj(nc, tc, wo_d, out_d, bo_sb, agouts)


def _emit_proj_attn(nc, tc, scale, xT_d, wq_d, wk_d, wv_d,
                    bq_sb, bk_sb, bv_sb, mask_sb, ones_sb, vv, dramp):
    # x chunks + projection weights resident only for this region; the
    # pool closes before the out-projection so its SBUF is reusable there.
    with tc.tile_pool(name="xw", bufs=1) as xwp:
        xsb = xwp.tile([128, HC, S], F16, name="xsb")
        wvsb = xwp.tile([128, HC, DG], F16, name="wvsb")
        wqsb = xwp.tile([128, HC, DG], F16, name="wqsb")
        wksb = xwp.tile([128, HC, DG], F16, name="wksb")
        # DMA order: wv + x strip0 interleaved (A2 starts ~1.5us in),
        # then x strips 1-3, then q/k weights (needed ~55us in).
        # First pair at single-chunk granularity so A2's first matmul
        # only waits on 256KB.
        for c in range(4):
            rows = slice(128 * c, 128 * c + 128)
            nc.sync.dma_start(
                wvsb[:, c:c + 1, :],
                wv_d[rows, :].rearrange("(c p) d -> p c d", p=128))
            nc.sync.dma_start(
                xsb[:, c:c + 1, 0:512],
                xT_d[rows, 0:512].rearrange("(c p) t -> p c t", p=128))
        for hb in range(1, 4):
            rows = slice(512 * hb, 512 * hb + 512)
            nc.sync.dma_start(
                wvsb[:, 4 * hb:4 * hb + 4, :],
                wv_d[rows, :].rearrange("(c p) d -> p c d", p=128))
            nc.sync.dma_start(
                xsb[:, 4 * hb:4 * hb + 4, 0:512],
                xT_d[rows, 0:512].rearrange("(c p) t -> p c t", p=128))
        for strip in range(1, SQ):
            cs = slice(512 * strip, 512 * strip + 512)
            for hb in range(4):
                rows = slice(512 * hb, 512 * hb + 512)
                nc.sync.dma_start(
                    xsb[:, 4 * hb:4 * hb + 4, cs],
                    xT_d[rows, cs].rearrange("(c p) t -> p c t", p=128))
        for hb in range(4):
            rows = slice(512 * hb, 512 * hb + 512)
            nc.sync.dma_start(
                wqsb[:, 4 * hb:4 * hb + 4, :],
                wq_d[rows, :].rearrange("(c p) d -> p c d", p=128))
            nc.sync.dma_start(
                wksb[:, 4 * hb:4 * hb + 4, :],
                wk_d[rows, :].rearrange("(c p) d -> p c d", p=128))

        # ---- A2: V projection (natural [s, d] layout, all heads) -------
        with tc.tile_pool(name="psV", bufs=2, space="PSUM") as psV:
            for strip in range(SQ):
                psv = [psV.tile([128, DG], FP, tag=f"psv{i}", name=f"psv{i}")
                       for i in range(4)]
                for c in range(HC):
                    for i in range(4):
                        tok = slice(512 * strip + 128 * i,
                                    512 * strip + 128 * i + 128)
                        nc.tensor.matmul(
                            psv[i][:],
                            xsb[:, c, tok],
                            wvsb[:, c, :],
                            start=(c == 0), stop=(c == HC - 1),
                        )
                for i in range(4):
                    nc.vector.tensor_add(vv[4 * strip + i][:], psv[i][:], bv_sb[:])

        # ---- per-head: Q/K projection + attention + AllGather ----------
        agouts = []
        with tc.tile_pool(name="qkt", bufs=1) as qktp, \
             tc.tile_pool(name="etp", bufs=5) as etp, \
             tc.tile_pool(name="atp", bufs=3) as atp, \
             tc.tile_pool(name="rbp", bufs=2) as rbp, \
             tc.tile_pool(name="psQK", bufs=1, space="PSUM") as psQK, \
             tc.tile_pool(name="psS", bufs=2, space="PSUM") as psS, \
             tc.tile_pool(name="psAV", bufs=2, space="PSUM") as psAV, \
             tc.tile_pool(name="psDN", bufs=2, space="PSUM") as psDN:

            for h in range(NL):
                hs = slice(128 * h, 128 * h + 128)
                qT = qktp.tile([128, S], F16, tag="qT", name="qT")
                kT = qktp.tile([128, S], F16, tag="kT", name="kT")

                # Q/K projection for this head (q chunks then k chunks per
                # strip, so each PSUM bank's drain hides under the other's
                # matmuls)
                for strip in range(SQ):
                    cs = slice(512 * strip, 512 * strip + 512)
                    psq = psQK.tile([128, 512], FP, tag="psq", name="psq")
                    for c in range(HC):
                        nc.tensor.matmul(
                            psq[:], wqsb[:, c, hs], xsb[:, c, cs],
                            start=(c == 0), stop=(c == HC - 1),
                        )
                    nc.scalar.activation(
                        qT[:, cs], psq[:],
                        mybir.ActivationFunctionType.Identity,
                        bias=bq_sb[:, h:h + 1],
                    )
                    psk = psQK.tile([128, 512], FP, tag="psk", name="psk")
                    for c in range(HC):
                        nc.tensor.matmul(
                            psk[:], wksb[:, c, hs], xsb[:, c, cs],
                            start=(c == 0), stop=(c == HC - 1),
                        )
                    nc.scalar.activation(
                        kT[:, cs], psk[:],
                        mybir.ActivationFunctionType.Identity,
                        bias=bk_sb[:, h:h + 1],
                    )

                # causal attention for this head
                for qs in range(SQ):
                    cs = slice(512 * qs, 512 * qs + 512)
                    nk = 4 * qs + 4
                    ps_av = psAV.tile([128, 512], FP, tag="ps_av", name="ps_av")
                    ps_dn = psDN.tile([128, 512], FP, tag="ps_dn", name="ps_dn")
                    for kt in range(nk):
                        j = kt - 4 * qs        # >=0: diagonal (partial) tile
                        off = 128 * j if j >= 0 else 0
                        qsl = slice(512 * qs + off, 512 * qs + 512)
                        w = 512 - off
                        ps_s = psS.tile([128, 512], FP, tag="ps_s", name="ps_s")
                        nc.tensor.matmul(
                            ps_s[:, off:512],
                            kT[:, 128 * kt:128 * kt + 128],
                            qT[:, qsl],
                            start=True, stop=True,
                        )
                        et = etp.tile([128, 512], BF, tag="et", name="et")
                        nc.scalar.activation(
                            et[:, off:512], ps_s[:, off:512],
                            mybir.ActivationFunctionType.Exp,
                            scale=scale,
                        )
                        if j >= 0:  # causal triangle on the leading 128 cols
                            nc.vector.tensor_mul(
                                et[:, off:off + 128], et[:, off:off + 128],
                                mask_sb[:])
                        nc.tensor.matmul(
                            ps_dn[:, off:512], ones_sb[:], et[:, off:512],
                            start=(kt == 0), stop=(kt == nk - 1),
                        )
                        nc.tensor.matmul(
                            ps_av[:, off:512], vv[kt][:, hs], et[:, off:512],
                            start=(kt == 0), stop=(kt == nk - 1),
                        )
                    # normalize: an = ps_av / denom (denom broadcast across
                    # partitions by the ones[128,128] lhsT of the dn matmul)
                    rb = rbp.tile([128, 512], FP, tag="rb", name="rb")
                    nc.vector.reciprocal_approx_fast(rb[:], ps_dn[:])
                    an = atp.tile([128, 512], F16, tag="an", name="an")
                    nc.vector.tensor_mul(an[:], ps_av[:], rb[:])
                    # per-strip AllGather: pieces ship while later strips /
                    # heads still compute, so the last (small) piece lands
                    # shortly after the final an instead of a whole-head
                    # collective gating the last out-projection.
                    agin = dramp.tile([128, 512], F16,
                                      tag=f"agin{h}_{qs}", name=f"agin{h}_{qs}")
                    nc.sync.dma_start(agin[:], an[:])
                    agout = dramp.tile([512, 512], F16,
                                       tag=f"agout{h}_{qs}", name=f"agout{h}_{qs}")
                    nc.gpsimd.collective_compute(
                        "AllGather",
                        mybir.AluOpType.bypass,
                        replica_groups=REPLICA_GROUPS,
                        ins=[agin.opt()],
                        outs=[agout.opt()],
                    )
                    agouts.append(agout)  # index h*SQ + qs
    return agouts


def _emit_outproj(nc, tc, wo_d, out_d, bo_sb, agouts):
    # 4 passes, each over its own head's AllGather pieces. Piece loads and
    # wo loads ride the scalar engine's DMA queue so they don't serialize
    # behind the attention an-DMAs on the sync queue; each [128,4,512]
    # piece staged once serves 4 token tiles.
    with tc.tile_pool(name="wop", bufs=1) as wop, \
         tc.tile_pool(name="oaccp", bufs=1) as oaccp, \
         tc.tile_pool(name="piecep", bufs=4) as piecep, \
         tc.tile_pool(name="outp", bufs=4) as outp, \
         tc.tile_pool(name="psO", bufs=4, space="PSUM") as psO:
        oacc = [oaccp.tile([128, DG], FP, tag=f"oacc{i}", name=f"oacc{i}")
                for i in range(ST_N)]
        wo4s = []
        for l in range(NL):
            wo4 = wop.tile([128, 4, DG], F16, tag=f"wo{l}", name=f"wo{l}")
            nc.scalar.dma_start(
                wo4[:],
                wo_d[512 * l:512 * l + 512, :].rearrange("(c p) d -> p c d", p=128))
            wo4s.append(wo4)
        for l in range(NL):
            last = (l == NL - 1)
            wo4 = wo4s[l]
            for ps in range(SQ):
                pc = piecep.tile([128, 4, 512], F16, tag="pc", name="pc")
                nc.scalar.dma_start(
                    pc[:],
                    agouts[SQ * l + ps][:].rearrange("(r p) t -> p r t", p=128))
                for si in range(4):
                    sti = 4 * ps + si
                    rs = slice(128 * sti, 128 * sti + 128)
                    lcl = slice(128 * si, 128 * si + 128)
                    ps_o = psO.tile([128, DG], FP, tag="ps_o", name="ps_o")
                    for r in range(4):
                        nc.tensor.matmul(
                            ps_o[:], pc[:, r, lcl], wo4[:, r, :],
                            start=(r == 0), stop=(r == 3),
                        )
                    if l == 0:
                        nc.vector.tensor_add(oacc[sti][:], ps_o[:], bo_sb[:])
                    elif not last:
                        nc.vector.tensor_add(oacc[sti][:], ps_o[:], oacc[sti][:])
                    else:
                        ob = outp.tile([128, DG], FP, tag="ob", name="ob")
                        nc.vector.tensor_add(ob[:], ps_o[:], oacc[sti][:])
                        nc.sync.dma_start(out_d[rs, :], ob[:])


def make_inputs(x, W_qkv, b_qkv, W_out, b_out):
    """Host-side sharding: per-core input dicts."""
    x = np.asarray(x, dtype=np.float32)
    W_qkv = np.asarray(W_qkv, dtype=np.float32)
    b_qkv = np.asarray(b_qkv, dtype=np.float32)
    W_out = np.asarray(W_out, dtype=np.float32)
    b_out = np.asarray(b_out, dtype=np.float32)

    # lower-triangle mask block: mask[i, t] = 1 iff t >= i
    tt = np.arange(128)[None, :]
    ii = np.arange(128)[:, None]
    mask = (tt >= ii).astype(ml_dtypes.bfloat16)

    WoT = W_out.T  # [h (d-in), h (n-out)]
    in_maps = []
    for c in range(NCORES):
        b, g = divmod(c, GROUPS)
        xT = np.ascontiguousarray(x[b].T.astype(np.float16))    # [h, s]
        wq = np.ascontiguousarray(
            W_qkv[DG * g:DG * (g + 1), :].T.astype(np.float16))
        wk = np.ascontiguousarray(
            W_qkv[H + DG * g:H + DG * (g + 1), :].T.astype(np.float16))
        wv = np.ascontiguousarray(
            W_qkv[2 * H + DG * g:2 * H + DG * (g + 1), :].T.astype(np.float16))
        bq = np.ascontiguousarray(
            b_qkv[DG * g:DG * (g + 1)].reshape(NL, 128).T)      # [128, nl]
        bk = np.ascontiguousarray(
            b_qkv[H + DG * g:H + DG * (g + 1)].reshape(NL, 128).T)
        bv = np.tile(b_qkv[2 * H + DG * g:2 * H + DG * (g + 1)][None, :], (128, 1))
        bo = np.tile(b_out[DG * g:DG * (g + 1)][None, :], (128, 1))
        # W_out^T rows permuted to the AllGather d-order:
        # ci = l*4 + r  ->  global head 4r + l (within this batch group)
        blocks = []
        for l in range(NL):
            for r in range(GROUPS):
                hh = NL * r + l  # head held as local-head l by group-rank r
                blocks.append(WoT[D * hh:D * (hh + 1), DG * g:DG * (g + 1)])
        wo = np.ascontiguousarray(
            np.concatenate(blocks, axis=0).astype(np.float16))  # [h, dg] fp16
        in_maps.append({
            "xT": xT, "wq": wq, "wk": wk, "wv": wv, "wo": wo,
            "bq": bq, "bk": bk,
            "bv": np.ascontiguousarray(bv), "bo": np.ascontiguousarray(bo),
            "mask": mask,
        })
    return in_maps


_NC_CACHE = {}


def _get_nc():
    if "nc" not in _NC_CACHE:
        _NC_CACHE["nc"] = build_nc()
    return _NC_CACHE["nc"]


def kernel(x, W_qkv, b_qkv, W_out, b_out):
    global LAST_EXEC_NS, LAST_RESULTS
    nc = _get_nc()
    in_maps = make_inputs(x, W_qkv, b_qkv, W_out, b_out)
    res = run_bass_kernel_spmd(
        nc, in_maps, core_ids=list(range(NCORES)), trace=TRACE)
    LAST_EXEC_NS = res.exec_time_ns
    LAST_RESULTS = res
    out = np.empty((B, S, H), dtype=np.float32)
    for c in range(NCORES):
        b, g = divmod(c, GROUPS)
        out[b, :, DG * g:DG * (g + 1)] = res.results[c]["out"]
    return out


# revision 19
# speedup vs baseline: 1.0288x; 1.0033x over previous
"""Trainium2 Bass kernel: fused multi-head causal self-attention block.

Computes, for x:(B,S,H), W_qkv:(3H,H), b_qkv:(3H,), W_out:(H,H), b_out:(H,):
    qkv = x @ W_qkv.T + b_qkv ; split into q,k,v heads (NH heads, D=H/NH)
    out = softmax(causal(q k^T / sqrt(D))) v   ; merge heads
    return out @ W_out.T + b_out

Sharding over 8 NeuronCores: DP(2 batches) x TP(4 head-groups).
Core c handles batch b=c//4, head group g=c%4 (heads 4g..4g+3).

Schedule per core:
  - x chunks + all weights DMA'd into SBUF once (fp16).
  - A2: V projection for all 4 local heads.
  - per head h: Q/K projection -> causal attention -> AllGather of A^T
    (fp16) within the 4-core batch group; collectives overlap the
    following heads' compute.
  - 4 out-projection passes at the end (each only needs its own
    AllGather, so the last collective hides under the first 3 passes).
Host does a pure concatenation of the per-core [S, 512] output slices.

All matmuls run 16-bit (fp16 operands; bf16 for exp-weights/V);
accumulation is fp32 in PSUM. Diagonal attention tiles are
width-restricted (causal) to skip fully-masked columns.
"""

import math

import numpy as np
import ml_dtypes

import concourse.bass as bass
import concourse.mybir as mybir
import concourse.tile as tile
from concourse import bacc
from concourse.bass_utils import run_bass_kernel_spmd

FP = mybir.dt.float32
FR = mybir.dt.float32r
F16 = mybir.dt.float16
BF = mybir.dt.bfloat16

# Full-size problem constants.
B, S, H, NH = 2, 2048, 2048, 16
D = 128
NCORES = 8
GROUPS = 4                  # head-groups per batch (TP degree)
REPLICA_GROUPS = [[0, 1, 2, 3], [4, 5, 6, 7]]

TRACE = False               # set by test harness to capture NTFF profile
LAST_EXEC_NS = None
LAST_RESULTS = None

NL = NH // GROUPS           # local heads per core (4)
DG = NL * D                 # per-core slice of the head dim (512)
HC = H // 128               # contraction chunks (16)
SQ = S // 512               # 512-wide strips (4)
ST_N = S // 128             # 128-row s tiles (16)


def build_nc():
    """Build the SPMD Bass program (identical on all 8 cores)."""
    scale = 1.0 / math.sqrt(D)

    nc = bacc.Bacc(
        "TRN2",
        target_bir_lowering=False,
        debug=False,
        enable_asserts=False,
        num_devices=NCORES,
    )

    # ---- I/O -----------------------------------------------------------
    xT_d = nc.dram_tensor("xT", [H, S], F16, kind="ExternalInput")
    wq_d = nc.dram_tensor("wq", [H, DG], F16, kind="ExternalInput")
    wk_d = nc.dram_tensor("wk", [H, DG], F16, kind="ExternalInput")
    wv_d = nc.dram_tensor("wv", [H, DG], F16, kind="ExternalInput")
    wo_d = nc.dram_tensor("wo", [H, DG], F16, kind="ExternalInput")
    bq_d = nc.dram_tensor("bq", [128, NL], FP, kind="ExternalInput")
    bk_d = nc.dram_tensor("bk", [128, NL], FP, kind="ExternalInput")
    bv_d = nc.dram_tensor("bv", [128, DG], FP, kind="ExternalInput")
    bo_d = nc.dram_tensor("bo", [128, DG], FP, kind="ExternalInput")
    mask_d = nc.dram_tensor("mask", [128, 128], BF, kind="ExternalInput")
    out_d = nc.dram_tensor("out", [S, DG], FP, kind="ExternalOutput")

    with tile.TileContext(nc) as tc:
        with tc.tile_pool(name="const", bufs=1) as constp:
            # constants ride the scalar DMA queue so the sync queue's first
            # transfers are the wv/x chunks the first matmul waits on
            mask_sb = constp.tile([128, 128], BF)
            nc.scalar.dma_start(mask_sb[:], mask_d[:])
            bq_sb = constp.tile([128, NL], FP)
            nc.scalar.dma_start(bq_sb[:], bq_d[:])
            bk_sb = constp.tile([128, NL], FP)
            nc.scalar.dma_start(bk_sb[:], bk_d[:])
            bv_sb = constp.tile([128, DG], FP)
            nc.scalar.dma_start(bv_sb[:], bv_d[:])
            bo_sb = constp.tile([128, DG], FP)
            nc.scalar.dma_start(bo_sb[:], bo_d[:])
            ones_sb = constp.tile([128, 128], BF)
            nc.vector.memset(ones_sb[:], 1.0)

            _emit_body(nc, tc, scale,
                       xT_d, wq_d, wk_d, wv_d, wo_d, out_d,
                       bq_sb, bk_sb, bv_sb, bo_sb, mask_sb, ones_sb)

    nc.compile()
    return nc


def _emit_body(nc, tc, scale,
               xT_d, wq_d, wk_d, wv_d, wo_d, out_d,
               bq_sb, bk_sb, bv_sb, bo_sb, mask_sb, ones_sb):
    with tc.tile_pool(name="vv", bufs=1) as vvp, \
         tc.tile_pool(name="dramp", bufs=1, space="DRAM") as dramp:
        vv = [vvp.tile([128, DG], BF, tag=f"v{t}", name=f"v{t}")
              for t in range(ST_N)]
        agouts = _emit_proj_attn(nc, tc, scale, xT_d, wq_d, wk_d, wv_d,
                                 bq_sb, bk_sb, bv_sb, mask_sb, ones_sb, vv,
                                 dramp)
        _emit_outproj(nc, tc, wo_d, out_d, bo_sb, agouts)


def _emit_proj_attn(nc, tc, scale, xT_d, wq_d, wk_d, wv_d,
                    bq_sb, bk_sb, bv_sb, mask_sb, ones_sb, vv, dramp):
    # x chunks + projection weights resident only for this region; the
    # pool closes before the out-projection so its SBUF is reusable there.
    with tc.tile_pool(name="xw", bufs=1) as xwp:
        xsb = xwp.tile([128, HC, S], F16, name="xsb")
        wvsb = xwp.tile([128, HC, DG], F16, name="wvsb")
        wqsb = xwp.tile([128, HC, DG], F16, name="wqsb")
        wksb = xwp.tile([128, HC, DG], F16, name="wksb")
        # DMA order: wv + x strip0 interleaved (A2 starts ~1.5us in),
        # then x strips 1-3, then q/k weights (needed ~55us in).
        # First pair at single-chunk granularity so A2's first matmul
        # only waits on 256KB.
        for c in range(4):
            rows = slice(128 * c, 128 * c + 128)
            nc.sync.dma_start(
                wvsb[:, c:c + 1, :],
                wv_d[rows, :].rearrange("(c p) d -> p c d", p=128))
            nc.sync.dma_start(
                xsb[:, c:c + 1, 0:512],
                xT_d[rows, 0:512].rearrange("(c p) t -> p c t", p=128))
        for hb in range(1, 4):
            rows = slice(512 * hb, 512 * hb + 512)
            nc.sync.dma_start(
                wvsb[:, 4 * hb:4 * hb + 4, :],
                wv_d[rows, :].rearrange("(c p) d -> p c d", p=128))
            nc.sync.dma_start(
                xsb[:, 4 * hb:4 * hb + 4, 0:512],
                xT_d[rows, 0:512].rearrange("(c p) t -> p c t", p=128))
        for strip in range(1, SQ):
            cs = slice(512 * strip, 512 * strip + 512)
            for hb in range(4):
                rows = slice(512 * hb, 512 * hb + 512)
                nc.sync.dma_start(
                    xsb[:, 4 * hb:4 * hb + 4, cs],
                    xT_d[rows, cs].rearrange("(c p) t -> p c t", p=128))
        for hb in range(4):
            rows = slice(512 * hb, 512 * hb + 512)
            nc.sync.dma_start(
                wqsb[:, 4 * hb:4 * hb + 4, :],
                wq_d[rows, :].rearrange("(c p) d -> p c d", p=128))
            nc.sync.dma_start(
                wksb[:, 4 * hb:4 * hb + 4, :],
                wk_d[rows, :].rearrange("(c p) d -> p c d", p=128))

        # ---- A2: V projection (natural [s, d] layout, all heads) -------
        with tc.tile_pool(name="psV", bufs=2, space="PSUM") as psV:
            for strip in range(SQ):
                psv = [psV.tile([128, DG], FP, tag=f"psv{i}", name=f"psv{i}")
                       for i in range(4)]
                for c in range(HC):
                    for i in range(4):
                        tok = slice(512 * strip + 128 * i,
                                    512 * strip + 128 * i + 128)
                        nc.tensor.matmul(
                            psv[i][:],
                            xsb[:, c, tok],
                            wvsb[:, c, :],
                            start=(c == 0), stop=(c == HC - 1),
                        )
                for i in range(4):
                    nc.vector.tensor_add(vv[4 * strip + i][:], psv[i][:], bv_sb[:])

        # ---- per-head: Q/K projection + attention + AllGather ----------
        agouts = []
        with tc.tile_pool(name="qkt", bufs=1) as qktp, \
             tc.tile_pool(name="etp", bufs=5) as etp, \
             tc.tile_pool(name="atp", bufs=3) as atp, \
             tc.tile_pool(name="rbp", bufs=2) as rbp, \
             tc.tile_pool(name="psQK", bufs=1, space="PSUM") as psQK, \
             tc.tile_pool(name="psS", bufs=2, space="PSUM") as psS, \
             tc.tile_pool(name="psAV", bufs=2, space="PSUM") as psAV, \
             tc.tile_pool(name="psDN", bufs=2, space="PSUM") as psDN:

            for h in range(NL):
                hs = slice(128 * h, 128 * h + 128)
                qT = qktp.tile([128, S], F16, tag="qT", name="qT")
                kT = qktp.tile([128, S], F16, tag="kT", name="kT")

                # Q/K projection for this head (q chunks then k chunks per
                # strip, so each PSUM bank's drain hides under the other's
                # matmuls)
                for strip in range(SQ):
                    cs = slice(512 * strip, 512 * strip + 512)
                    psq = psQK.tile([128, 512], FP, tag="psq", name="psq")
                    for c in range(HC):
                        nc.tensor.matmul(
                            psq[:], wqsb[:, c, hs], xsb[:, c, cs],
                            start=(c == 0), stop=(c == HC - 1),
                        )
                    nc.scalar.activation(
                        qT[:, cs], psq[:],
                        mybir.ActivationFunctionType.Identity,
                        bias=bq_sb[:, h:h + 1],
                    )
                    psk = psQK.tile([128, 512], FP, tag="psk", name="psk")
                    for c in range(HC):
                        nc.tensor.matmul(
                            psk[:], wksb[:, c, hs], xsb[:, c, cs],
                            start=(c == 0), stop=(c == HC - 1),
                        )
                    nc.scalar.activation(
                        kT[:, cs], psk[:],
                        mybir.ActivationFunctionType.Identity,
                        bias=bk_sb[:, h:h + 1],
                    )

                # causal attention for this head
                for qs in range(SQ):
                    cs = slice(512 * qs, 512 * qs + 512)
                    nk = 4 * qs + 4
                    ps_av = psAV.tile([128, 512], FP, tag="ps_av", name="ps_av")
                    ps_dn = psDN.tile([128, 512], FP, tag="ps_dn", name="ps_dn")
                    for kt in range(nk):
                        j = kt - 4 * qs        # >=0: diagonal (partial) tile
                        off = 128 * j if j >= 0 else 0
                        qsl = slice(512 * qs + off, 512 * qs + 512)
                        w = 512 - off
                        ps_s = psS.tile([128, 512], FP, tag="ps_s", name="ps_s")
                        nc.tensor.matmul(
                            ps_s[:, off:512],
                            kT[:, 128 * kt:128 * kt + 128],
                            qT[:, qsl],
                            start=True, stop=True,
                        )
                        et = etp.tile([128, 512], BF, tag="et", name="et")
                        nc.scalar.activation(
                            et[:, off:512], ps_s[:, off:512],
                            mybir.ActivationFunctionType.Exp,
                            scale=scale,
                        )
                        if j >= 0:  # causal triangle on the leading 128 cols
                            nc.vector.tensor_mul(
                                et[:, off:off + 128], et[:, off:off + 128],
                                mask_sb[:])
                        nc.tensor.matmul(
                            ps_dn[:, off:512], ones_sb[:], et[:, off:512],
                            start=(kt == 0), stop=(kt == nk - 1),
                        )
                        nc.tensor.matmul(
                            ps_av[:, off:512], vv[kt][:, hs], et[:, off:512],
                            start=(kt == 0), stop=(kt == nk - 1),
                        )
                    # normalize: an = ps_av / denom (denom broadcast across
                    # partitions by the ones[128,128] lhsT of the dn matmul)
                    rb = rbp.tile([128, 512], FP, tag="rb", name="rb")
                    nc.vector.reciprocal_approx_fast(rb[:], ps_dn[:])
                    an = atp.tile([128, 512], F16, tag="an", name="an")
                    nc.vector.tensor_mul(an[:], ps_av[:], rb[:])
                    # per-strip AllGather: pieces ship while later strips /
                    # heads still compute, so the last (small) piece lands
                    # shortly after the final an instead of a whole-head
                    # collective gating the last out-projection.
                    agin = dramp.tile([128, 512], F16,
                                      tag=f"agin{h}_{qs}", name=f"agin{h}_{qs}")
                    nc.sync.dma_start(agin[:], an[:])
                    agout = dramp.tile([512, 512], F16,
                                       tag=f"agout{h}_{qs}", name=f"agout{h}_{qs}")
                    nc.gpsimd.collective_compute(
                        "AllGather",
                        mybir.AluOpType.bypass,
                        replica_groups=REPLICA_GROUPS,
                        ins=[agin.opt()],
                        outs=[agout.opt()],
                    )
                    agouts.append(agout)  # index h*SQ + qs
    return agouts


def _emit_outproj(nc, tc, wo_d, out_d, bo_sb, agouts):
    # 4 passes, each over its own head's AllGather pieces. Piece loads and
    # wo loads ride the scalar engine's DMA queue so they don't serialize
    # behind the attention an-DMAs on the sync queue; each [128,4,512]
    # piece staged once serves 4 token tiles.
    with tc.tile_pool(name="wop", bufs=1) as wop, \
         tc.tile_pool(name="oaccp", bufs=1) as oaccp, \
         tc.tile_pool(name="piecep", bufs=4) as piecep, \
         tc.tile_pool(name="outp", bufs=4) as outp, \
         tc.tile_pool(name="psO", bufs=4, space="PSUM") as psO:
        oacc = [oaccp.tile([128, DG], FP, tag=f"oacc{i}", name=f"oacc{i}")
                for i in range(ST_N)]
        wo4s = [wop.tile([128, 4, DG], F16, tag=f"wo{l}", name=f"wo{l}")
                for l in range(NL)]

        def load_wo(l):
            nc.scalar.dma_start(
                wo4s[l][:],
                wo_d[512 * l:512 * l + 512, :].rearrange("(c p) d -> p c d", p=128))

        # scalar-queue order: wo4[0], then pass-0's pieces (so the first
        # out-proj matmul isn't queued behind all four weight loads), then
        # the remaining weights.
        load_wo(0)
        pcs0 = []
        for ps in range(SQ):
            pc = piecep.tile([128, 4, 512], F16, tag="pc", name="pc")
            nc.scalar.dma_start(
                pc[:], agouts[ps][:].rearrange("(r p) t -> p r t", p=128))
            pcs0.append(pc)
        for l in range(1, NL):
            load_wo(l)
        for l in range(NL):
            last = (l == NL - 1)
            wo4 = wo4s[l]
            for ps in range(SQ):
                if l == 0:
                    pc = pcs0[ps]
                else:
                    pc = piecep.tile([128, 4, 512], F16, tag="pc", name="pc")
                    nc.scalar.dma_start(
                        pc[:],
                        agouts[SQ * l + ps][:].rearrange("(r p) t -> p r t", p=128))
                for si in range(4):
                    sti = 4 * ps + si
                    rs = slice(128 * sti, 128 * sti + 128)
                    lcl = slice(128 * si, 128 * si + 128)
                    ps_o = psO.tile([128, DG], FP, tag="ps_o", name="ps_o")
                    for r in range(4):
                        nc.tensor.matmul(
                            ps_o[:], pc[:, r, lcl], wo4[:, r, :],
                            start=(r == 0), stop=(r == 3),
                        )
                    if l == 0:
                        nc.vector.tensor_add(oacc[sti][:], ps_o[:], bo_sb[:])
                    elif not last:
                        nc.vector.tensor_add(oacc[sti][:], ps_o[:], oacc[sti][:])
                    else:
                        ob = outp.tile([128, DG], FP, tag="ob", name="ob")
                        nc.vector.tensor_add(ob[:], ps_o[:], oacc[sti][:])
                        nc.sync.dma_start(out_d[rs, :], ob[:])


def make_inputs(x, W_qkv, b_qkv, W_out, b_out):
    """Host-side sharding: per-core input dicts."""
    x = np.asarray(x, dtype=np.float32)
    W_qkv = np.asarray(W_qkv, dtype=np.float32)
    b_qkv = np.asarray(b_qkv, dtype=np.float32)
    W_out = np.asarray(W_out, dtype=np.float32)
    b_out = np.asarray(b_out, dtype=np.float32)

    # lower-triangle mask block: mask[i, t] = 1 iff t >= i
    tt = np.arange(128)[None, :]
    ii = np.arange(128)[:, None]
    mask = (tt >= ii).astype(ml_dtypes.bfloat16)

    WoT = W_out.T  # [h (d-in), h (n-out)]
    in_maps = []
    for c in range(NCORES):
        b, g = divmod(c, GROUPS)
        xT = np.ascontiguousarray(x[b].T.astype(np.float16))    # [h, s]
        wq = np.ascontiguousarray(
            W_qkv[DG * g:DG * (g + 1), :].T.astype(np.float16))
        wk = np.ascontiguousarray(
            W_qkv[H + DG * g:H + DG * (g + 1), :].T.astype(np.float16))
        wv = np.ascontiguousarray(
            W_qkv[2 * H + DG * g:2 * H + DG * (g + 1), :].T.astype(np.float16))
        bq = np.ascontiguousarray(
            b_qkv[DG * g:DG * (g + 1)].reshape(NL, 128).T)      # [128, nl]
        bk = np.ascontiguousarray(
            b_qkv[H + DG * g:H + DG * (g + 1)].reshape(NL, 128).T)
        bv = np.tile(b_qkv[2 * H + DG * g:2 * H + DG * (g + 1)][None, :], (128, 1))
        bo = np.tile(b_out[DG * g:DG * (g + 1)][None, :], (128, 1))
        # W_out^T rows permuted to the AllGather d-order:
        # ci = l*4 + r  ->  global head 4r + l (within this batch group)
        blocks = []
        for l in range(NL):
            for r in range(GROUPS):
                hh = NL * r + l  # head held as local-head l by group-rank r
                blocks.append(WoT[D * hh:D * (hh + 1), DG * g:DG * (g + 1)])
        wo = np.ascontiguousarray(
            np.concatenate(blocks, axis=0).astype(np.float16))  # [h, dg] fp16
        in_maps.append({
            "xT": xT, "wq": wq, "wk": wk, "wv": wv, "wo": wo,
            "bq": bq, "bk": bk,
            "bv": np.ascontiguousarray(bv), "bo": np.ascontiguousarray(bo),
            "mask": mask,
        })
    return in_maps


_NC_CACHE = {}


def _get_nc():
    if "nc" not in _NC_CACHE:
        _NC_CACHE["nc"] = build_nc()
    return _NC_CACHE["nc"]


def kernel(x, W_qkv, b_qkv, W_out, b_out):
    global LAST_EXEC_NS, LAST_RESULTS
    nc = _get_nc()
    in_maps = make_inputs(x, W_qkv, b_qkv, W_out, b_out)
    res = run_bass_kernel_spmd(
        nc, in_maps, core_ids=list(range(NCORES)), trace=TRACE)
    LAST_EXEC_NS = res.exec_time_ns
    LAST_RESULTS = res
    out = np.empty((B, S, H), dtype=np.float32)
    for c in range(NCORES):
        b, g = divmod(c, GROUPS)
        out[b, :, DG * g:DG * (g + 1)] = res.results[c]["out"]
    return out
